# revision 29
# baseline (speedup 1.0000x reference)
"""Modulated Conv2D (StyleGAN2-style) Trainium2 Bass kernel.

Problem shapes (hardcoded):
  x: [16, 256, 64, 64] f32    y: [16, 512] f32
  weights: [256, 256, 3, 3]   bias: [256]
  style_w: [256, 512]         style_b: [256]
  out: [16, 256, 64, 64] f32

Two math transformations, both folded on the host while sharding:

1. Modulation/demodulation folded into the weights (style/winv are ~4 MMACs
   of the 77-GFLOP problem):
     out[b,o] = conv(x[b], w * style[b,i] * winv[b,o])[o] + bias[o]
     winv[b,o] = 1/sqrt(sum_i W2[o,i] * style[b,i]^2 + eps)

2. 1-D Winograd F(2,3) along the image width: each 3x3 conv row-tap becomes
   4 transform channels producing 2 output columns per tile, cutting PE
   columns 1.5x (12 tap-channels x 32 tiles vs 9 taps x 64 cols).  The
   weight transform U = G w (G = [[1,0,0],[.5,.5,.5],[.5,-.5,.5],[0,0,1]])
   is folded into the host weight prep; the input transform V = B^T d
   (V0=d0-d2, V1=d1+d2, V2=d2-d1, V3=d1-d3 over even/odd column pairs) runs
   on the DVE; the output transform (out_even = M0+M1+M2, out_odd =
   M1-M2-M3) runs on the DVE reading PSUM; ScalarE adds bias on a final
   copy; row taps (dy) accumulate in PSUM as before.

The device kernel: x (bf16, host-cast) DMAs straight into zero-row-padded
SBUF tiles (columns unpadded -> fully contiguous line-rate DMA; column
edges handled by special-cased V channels at tile 0/31), PE streams 768
256-col matmuls back-to-back, everything else hides in their shadow.

Sharding: data-parallel over batch, 2 samples per core across 8 cores.
Inputs stream over both HWDGE rings (order = priority); a dozen junk
matmuls up front lift the HAM clock gate before the conv arrives.
"""

import numpy as np
import ml_dtypes

import concourse.bass as bass
import concourse.tile as tile
from concourse import bacc, mybir
from concourse import bass_utils

EPS = 1e-8
P = 128
B_LOC = 2          # samples per core
CIN, COUT = 256, 256
NI, NO = CIN // P, COUT // P   # 2, 2
S = 512
H = W = 64
NT = 4             # Winograd F(2,3) transform channels
NJ = W // 2        # 32 column tiles (2 output columns each)
HP = H + 2         # zero-padded rows (columns are not padded)
N_CORES = 8
ROWS_A = 34        # first-chunk rows of the b=1 x load split
N_WARM_MM = 18     # junk matmuls to lift the HAM clock gate before the conv
WARM_COLS = 512    # columns per junk matmul (paces the warm-up)
V_REST_GPS = False  # non-critical V transform bands on GPSIMD
QU_GPS = True       # fold q/u tensor-tensor ops on GPSIMD

F32 = mybir.dt.float32
BF16 = mybir.dt.bfloat16
AF = mybir.ActivationFunctionType


def build_conv2dmod(nc):
    xbf = nc.dram_tensor("xbf", [B_LOC, CIN, H, W], BF16, kind="ExternalInput")
    # Winograd-transformed folded weights: [b, i, dy, t, o]
    wu = nc.dram_tensor("wu", [B_LOC, CIN, 3, NT, COUT], BF16,
                        kind="ExternalInput")
    bias_col = nc.dram_tensor("bias_col", [P, NO], F32, kind="ExternalInput")
    out = nc.dram_tensor("out", [B_LOC, COUT, H, W], F32, kind="ExternalOutput")

    with tile.TileContext(nc) as tc:
        with (
            tc.tile_pool(name="consts", bufs=1) as consts,
            tc.tile_pool(name="xs_pool", bufs=1) as xs_pool,
            tc.tile_pool(name="v_pool", bufs=1) as v_pool,
            tc.tile_pool(name="out_pool", bufs=2) as out_pool,
            tc.tile_pool(name="psum", bufs=1, space="PSUM") as psum,
        ):
            bias_sb = consts.tile([P, NO], F32)
            nc.sync.dma_start(bias_sb[:], bias_col.ap())

            # x tiles: row-padded only, so both the DMA source and the SBUF
            # destination are fully contiguous per partition (line-rate DMA)
            xs = {}
            vt = {}
            for b in range(B_LOC):
                for it in range(NI):
                    xs[(b, it)] = xs_pool.tile([P, HP, W], BF16,
                                               name=f"xs{b}_{it}", tag=f"xs{b}_{it}")
                    vt[(b, it)] = v_pool.tile([P, HP, NT, NJ], BF16,
                                              name=f"v{b}_{it}", tag=f"v{b}_{it}")

            def load_x(eng, b, it, r):
                eng.dma_start(xs[(b, it)][:, r.start + 1:r.stop + 1, :],
                              xbf.ap()[b, it * P:(it + 1) * P, r, :])

            w_sb = {}
            for b in range(B_LOC):
                for it in range(NI):
                    w_sb[(b, it)] = consts.tile([P, 3, NT, COUT], BF16,
                                                name=f"w{b}_{it}", tag=f"w{b}_{it}")

            def load_w(eng, b, it, d0, d1):
                eng.dma_start(w_sb[(b, it)][:, d0:d1], wf_ap(b, it, d0, d1))

            def wf_ap(b, it, d0, d1):
                return wu.ap()[b, it * P:(it + 1) * P, d0:d1]

            # load order = priority; sample-0 x in three row slices so the
            # first conv sub-block can start as early as possible
            RB = (slice(0, 17), slice(17, ROWS_A), slice(ROWS_A, H))
            load_x(nc.sync, 0, 0, RB[0])
            load_w(nc.scalar, 0, 0, 0, 2)
            load_x(nc.scalar, 0, 1, RB[0])
            load_x(nc.sync, 0, 0, RB[1])
            load_w(nc.scalar, 0, 0, 2, 3)
            load_w(nc.sync, 0, 1, 0, 2)
            load_x(nc.scalar, 0, 1, RB[1])
            load_w(nc.sync, 0, 1, 2, 3)
            load_x(nc.sync, 0, 0, RB[2])
            load_x(nc.scalar, 0, 1, RB[2])
            load_x(nc.scalar, 1, 0, slice(0, ROWS_A))
            load_x(nc.scalar, 1, 0, slice(ROWS_A, H))
            load_x(nc.scalar, 1, 1, slice(0, ROWS_A))
            load_x(nc.scalar, 1, 1, slice(ROWS_A, H))
            load_w(nc.sync, 1, 0, 0, 3)
            load_w(nc.scalar, 1, 1, 0, 3)

            # ---------- ACT func-table warm ----------
            lafs_warm = consts.tile([P, 1], F32)
            nc.scalar.activation(lafs_warm[:], bias_sb[:, 0:1], AF.Identity,
                                 bias=bias_sb[:, 1:2])

            # ---------- PE warm-up: junk matmuls while x streams in ----------
            warm_src = consts.tile([P, WARM_COLS], BF16)
            nc.gpsimd.memset(warm_src[:], 0.0)
            warm_ps = psum.tile([1, WARM_COLS], F32, name="warm_ps", tag="ch0_0")
            for _ in range(N_WARM_MM):
                nc.tensor.matmul(warm_ps[:], warm_src[:, 0:1], warm_src[:],
                                 start=True, stop=True)

            for b in range(B_LOC):
                for it in range(NI):
                    nc.gpsimd.memset(xs[(b, it)][:, 0, :], 0.0)
                    nc.gpsimd.memset(xs[(b, it)][:, HP - 1, :], 0.0)

            # ---------- input transform (DVE): V = B^T d per row band -------
            # tile j covers out cols 2j,2j+1; d0..d3 = x cols 2j-1..2j+2
            # V0=d0-d2  V1=d1+d2  V2=d2-d1  V3=d1-d3
            # edges: j=0 has d0=0 -> V0=-d2 ;  j=31 has d3=0 -> V3=d1
            def make_v(eng, b, it, r):
                x_ = xs[(b, it)]
                v_ = vt[(b, it)]
                rr = slice(r.start + 1, r.stop + 1) if r is not None else slice(0, HP)
                xv = x_[:, rr, :].rearrange("p r (j two) -> p r j two", two=2)
                d1 = xv[:, :, :, 0]                      # cols 0,2,..62   j=0..31
                d2 = xv[:, :, :, 1]                      # cols 1,3,..63   j=0..31
                xo = x_[:, rr, 1:63].rearrange("p r (j two) -> p r j two", two=2)
                d0i = xo[:, :, :, 0]                     # cols 1,3,..61   j=1..31
                d3i = x_[:, rr, 2:64].rearrange(
                    "p r (j two) -> p r j two", two=2)[:, :, :, 0]  # cols 2..62, j=0..30
                sub, add = mybir.AluOpType.subtract, mybir.AluOpType.add
                v_sl = v_[:, rr]
                eng.tensor_tensor(v_sl[:, :, 0, 1:NJ], d0i, d2[:, :, 1:], sub)
                eng.tensor_scalar_mul(v_sl[:, :, 0, 0:1], d2[:, :, 0:1], -1.0)
                eng.tensor_tensor(v_sl[:, :, 1, :], d1, d2, add)
                eng.tensor_tensor(v_sl[:, :, 2, :], d2, d1, sub)
                eng.tensor_tensor(v_sl[:, :, 3, 0:NJ - 1],
                                  d1[:, :, 0:NJ - 1], d3i, sub)
                eng.tensor_copy(v_sl[:, :, 3, NJ - 1:NJ], d1[:, :, NJ - 1:NJ])

            # pad rows: compute V over the pad rows too (xs pads are zero,
            # so V is zero there) by extending the first/last band.  The
            # startup-critical first bands run on the DVE; the rest on the
            # otherwise-idle GPSIMD.
            make_v(nc.vector, 0, 0, slice(-1, 17))
            make_v(nc.vector, 0, 1, slice(-1, 17))
            make_v(nc.vector, 0, 0, RB[1])
            make_v(nc.vector, 0, 1, RB[1])
            _veng = nc.gpsimd if V_REST_GPS else nc.vector
            make_v(_veng, 0, 0, slice(ROWS_A, H + 1))
            make_v(_veng, 0, 1, slice(ROWS_A, H + 1))

            # ---------- main conv block ----------
            # per 8-row chunk and transform channel t: accumulate over
            # (it, dy) into a [P, 8, NJ] psum tile, then the DVE folds
            # t-channels into even/odd output columns of the oh tile
            def mm_block(b, ot, r0, nchunks, ctag0=0):
                pcs = []
                for c in range(nchunks):
                    for t in range(NT):
                        pc = psum.tile([P, 8, NJ], F32,
                                       name=f"pc{b}{ot}{r0}_{c}_{t}",
                                       tag=f"ch{(ctag0 + c) % 2}_{t}")
                        pcs.append(pc)
                        for g, (it, dy) in enumerate(
                                (i, d) for i in range(NI) for d in range(3)):
                            lhsT = w_sb[(b, it)][:, dy, t, ot * P:(ot + 1) * P]
                            rs = r0 + c * 8 + dy
                            nc.tensor.matmul(
                                pc[:], lhsT, vt[(b, it)][:, rs:rs + 8, t, :],
                                start=(g == 0), stop=(g == 5),
                            )
                return pcs

            def out_block(b, ot, r0, pcs, ring=None):
                n = len(pcs) // NT
                oh = out_pool.tile([P, 8 * n, W], F32, name=f"oh{b}{ot}{r0}",
                                   tag="oh")
                sub, add = mybir.AluOpType.subtract, mybir.AluOpType.add
                bi = bias_sb[:, ot:ot + 1]
                for c in range(n):
                    m = pcs[c * NT:(c + 1) * NT]
                    # ACT drains each t-channel (single PSUM read per op);
                    # DVE folds in SBUF only
                    st = out_pool.tile([P, NT, 8, NJ], F32,
                                       name=f"st{b}{ot}{r0}_{c}", tag=f"st{c % 2}")
                    for t in range(NT):
                        nc.scalar.activation(st[:, t], m[t][:], AF.Identity)
                    sq = out_pool.tile([P, 2, 8, NJ], F32,
                                       name=f"sq{b}{ot}{r0}_{c}", tag=f"sq{c % 2}")
                    ov = oh[:, c * 8:(c + 1) * 8, :].rearrange(
                        "p r (j two) -> p r j two", two=2)
                    # out_even = ((m0+m1) + bias) + m2
                    # out_odd  = ((m1-m2) + bias) - m3
                    _qeng = nc.gpsimd if QU_GPS else nc.vector
                    _qeng.tensor_tensor(sq[:, 0], st[:, 0], st[:, 1], add)
                    nc.vector.scalar_tensor_tensor(ov[:, :, :, 0], sq[:, 0], bi,
                                                   st[:, 2], add, add)
                    _qeng.tensor_tensor(sq[:, 1], st[:, 1], st[:, 2], sub)
                    nc.vector.scalar_tensor_tensor(ov[:, :, :, 1], sq[:, 1], bi,
                                                   st[:, 3], add, sub)
                (ring or nc.sync).dma_start(
                    out.ap()[b, ot * P:(ot + 1) * P, r0:r0 + 8 * n, :], oh[:])

            # first two sub-blocks are 16-row so the conv starts as soon as
            # the first 17-row x slice lands
            out_block(0, 0, 0, mm_block(0, 0, 0, 2, ctag0=0))
            out_block(0, 0, 16, mm_block(0, 0, 16, 2, ctag0=2))
            out_block(0, 0, 32, mm_block(0, 0, 32, 4))
            out_block(0, 1, 0, mm_block(0, 1, 0, 4))

            # sample-1 input transforms (GPSIMD), emitted mid-sample-0 so
            # they are ready well before the b=1 blocks
            for it in range(NI):
                make_v(nc.gpsimd if V_REST_GPS else nc.vector, 1, it, slice(-1, H + 1))

            out_block(0, 1, 32, mm_block(0, 1, 32, 4))
            for half in range(2):
                out_block(1, 0, half * 32, mm_block(1, 0, half * 32, 4))
            out_block(1, 1, 0, mm_block(1, 1, 0, 4))
            # final blocks shrink so the drain tail is short, stores split
            # across both rings
            out_block(1, 1, 32, mm_block(1, 1, 32, 2, ctag0=0))
            out_block(1, 1, 48, mm_block(1, 1, 48, 1, ctag0=2), ring=nc.scalar)
            out_block(1, 1, 56, mm_block(1, 1, 56, 1, ctag0=3))
    return nc


_CACHED_NC = None
_PREP_JIT = None


def _get_nc():
    global _CACHED_NC
    if _CACHED_NC is None:
        nc = bacc.Bacc("TRN2", target_bir_lowering=False, debug=False,
                       num_devices=N_CORES)
        build_conv2dmod(nc)
        nc.compile()
        _CACHED_NC = nc
    return _CACHED_NC


def _get_prep():
    """jit'd host-side prep on jax-cpu (multithreaded): x -> bf16 and the
    folded, Winograd-transformed per-sample conv weights
    wu[b,i,dy,t,o] = G @ (w*style*winv) in bf16."""
    global _PREP_JIT
    if _PREP_JIT is None:
        import jax
        import jax.numpy as jnp

        cpu = jax.devices("cpu")[0]

        def _prep(x, y, weights, bias, style_w, style_b):
            style = y @ style_w.T + style_b                       # [B, i]
            w2 = jnp.sum(weights * weights, axis=(2, 3))          # [o, i]
            sigma = (style * style) @ w2.T                        # [B, o]
            winv = 1.0 / jnp.sqrt(sigma + EPS)                    # [B, o]
            wfull = (weights[None] * style[:, None, :, None, None]
                     * winv[:, :, None, None, None])              # [B,o,i,dy,dx]
            w0 = wfull[..., 0]
            w1 = wfull[..., 1]
            w2_ = wfull[..., 2]
            u = jnp.stack([w0, 0.5 * (w0 + w1 + w2_),
                           0.5 * (w0 - w1 + w2_), w2_], axis=-1)  # [B,o,i,dy,t]
            wu = u.transpose(0, 2, 3, 4, 1).astype(jnp.bfloat16)  # [B,i,dy,t,o]
            return x.astype(jnp.bfloat16), wu

        _PREP_JIT = (jax.jit(_prep, device=cpu), cpu)
    return _PREP_JIT


def kernel(x, y, weights, bias, style_w, style_b, _trace=False):
    x = np.asarray(x, dtype=np.float32)
    y = np.asarray(y, dtype=np.float32)
    weights = np.asarray(weights, dtype=np.float32)
    bias = np.asarray(bias, dtype=np.float32)
    style_w = np.asarray(style_w, dtype=np.float32)
    style_b = np.asarray(style_b, dtype=np.float32)

    prep, cpu = _get_prep()
    import jax
    with jax.default_device(cpu):
        xbf, wuj = prep(x, y, weights, bias, style_w, style_b)
        xbf = np.asarray(xbf)
        wun = np.asarray(wuj)
    bias_c = np.ascontiguousarray(bias.reshape(NO, P).T)           # [p, oo]

    nc = _get_nc()
    in_maps = []
    for c in range(N_CORES):
        sl = slice(c * B_LOC, (c + 1) * B_LOC)
        in_maps.append({
            "xbf": np.ascontiguousarray(xbf[sl]),
            "wu": np.ascontiguousarray(wun[sl]),
            "bias_col": bias_c,
        })
    res = bass_utils.run_bass_kernel_spmd(
        nc, in_maps, core_ids=list(range(N_CORES)), trace=_trace
    )
    out = np.concatenate([r["out"] for r in res.results], axis=0)
    if _trace:
        kernel.last_results = res
    return out


# revision 30
# speedup vs baseline: 1.1539x; 1.1539x over previous
"""Modulated Conv2D (StyleGAN2-style) Trainium2 Bass kernel.

Problem shapes (hardcoded):
  x: [16, 256, 64, 64] f32    y: [16, 512] f32
  weights: [256, 256, 3, 3]   bias: [256]
  style_w: [256, 512]         style_b: [256]
  out: [16, 256, 64, 64] f32

Two math transformations, both folded on the host while sharding:

1. Modulation/demodulation folded into the weights (style/winv are ~4 MMACs
   of the 77-GFLOP problem):
     out[b,o] = conv(x[b], w * style[b,i] * winv[b,o])[o] + bias[o]
     winv[b,o] = 1/sqrt(sum_i W2[o,i] * style[b,i]^2 + eps)

2. 1-D Winograd F(2,3) along the image width: each 3x3 conv row-tap becomes
   4 transform channels producing 2 output columns per tile, cutting PE
   columns 1.5x (12 tap-channels x 32 tiles vs 9 taps x 64 cols).  The
   weight transform U = G w (G = [[1,0,0],[.5,.5,.5],[.5,-.5,.5],[0,0,1]])
   is folded into the host weight prep; the input transform V = B^T d
   (V0=d0-d2, V1=d1+d2, V2=d2-d1, V3=d1-d3 over even/odd column pairs) runs
   on the DVE; the output transform (out_even = M0+M1+M2, out_odd =
   M1-M2-M3) runs on the DVE reading PSUM; ScalarE adds bias on a final
   copy; row taps (dy) accumulate in PSUM as before.

The device kernel: x (bf16, host-cast) DMAs straight into zero-row-padded
SBUF tiles (columns unpadded -> fully contiguous line-rate DMA; column
edges handled by special-cased V channels at tile 0/31), PE streams 768
256-col matmuls back-to-back, everything else hides in their shadow.

Sharding: data-parallel over batch, 2 samples per core across 8 cores.
Inputs stream over both HWDGE rings (order = priority); a dozen junk
matmuls up front lift the HAM clock gate before the conv arrives.
"""

import numpy as np
import ml_dtypes

import concourse.bass as bass
import concourse.tile as tile
from concourse import bacc, mybir
from concourse import bass_utils

EPS = 1e-8
P = 128
B_LOC = 2          # samples per core
CIN, COUT = 256, 256
NI, NO = CIN // P, COUT // P   # 2, 2
S = 512
H = W = 64
NT = 4             # Winograd F(2,3) transform channels
NJ = W // 2        # 32 column tiles (2 output columns each)
HP = H + 2         # zero-padded rows (columns are not padded)
N_CORES = 8
ROWS_A = 34        # first-chunk rows of the b=1 x load split
N_WARM_MM = 18     # junk matmuls to lift the HAM clock gate before the conv
WARM_COLS = 512    # columns per junk matmul (paces the warm-up)
V_REST_GPS = False  # non-critical V transform bands on GPSIMD
QU_GPS = False      # fold q/u tensor-tensor ops on GPSIMD

F32 = mybir.dt.float32
BF16 = mybir.dt.bfloat16
AF = mybir.ActivationFunctionType


def build_conv2dmod(nc):
    xbf = nc.dram_tensor("xbf", [B_LOC, CIN, H, W], BF16, kind="ExternalInput")
    # Winograd-transformed folded weights: [b, i, dy, t, o]
    wu = nc.dram_tensor("wu", [B_LOC, CIN, 3, NT, COUT], BF16,
                        kind="ExternalInput")
    bias_col = nc.dram_tensor("bias_col", [P, NO], F32, kind="ExternalInput")
    out = nc.dram_tensor("out", [B_LOC, COUT, H, W], F32, kind="ExternalOutput")

    with tile.TileContext(nc) as tc:
        with (
            tc.tile_pool(name="consts", bufs=1) as consts,
            tc.tile_pool(name="xs_pool", bufs=1) as xs_pool,
            tc.tile_pool(name="v_pool", bufs=1) as v_pool,
            tc.tile_pool(name="out_pool", bufs=2) as out_pool,
            tc.tile_pool(name="psum", bufs=1, space="PSUM") as psum,
        ):
            bias_sb = consts.tile([P, NO], F32)
            nc.sync.dma_start(bias_sb[:], bias_col.ap())

            # x tiles: row-padded only, so both the DMA source and the SBUF
            # destination are fully contiguous per partition (line-rate DMA)
            xs = {}
            vt = {}
            for b in range(B_LOC):
                for it in range(NI):
                    xs[(b, it)] = xs_pool.tile([P, HP, W], BF16,
                                               name=f"xs{b}_{it}", tag=f"xs{b}_{it}")
                    vt[(b, it)] = v_pool.tile([P, HP, NT, NJ], BF16,
                                              name=f"v{b}_{it}", tag=f"v{b}_{it}")

            def load_x(eng, b, it, r):
                eng.dma_start(xs[(b, it)][:, r.start + 1:r.stop + 1, :],
                              xbf.ap()[b, it * P:(it + 1) * P, r, :])

            w_sb = {}
            for b in range(B_LOC):
                for it in range(NI):
                    w_sb[(b, it)] = consts.tile([P, 3, NT, COUT], BF16,
                                                name=f"w{b}_{it}", tag=f"w{b}_{it}")

            def load_w(eng, b, it, d0, d1):
                eng.dma_start(w_sb[(b, it)][:, d0:d1], wf_ap(b, it, d0, d1))

            def wf_ap(b, it, d0, d1):
                return wu.ap()[b, it * P:(it + 1) * P, d0:d1]

            # load order = priority; sample-0 x in three row slices so the
            # first conv sub-block can start as early as possible
            RB = (slice(0, 17), slice(17, ROWS_A), slice(ROWS_A, H))
            load_x(nc.sync, 0, 0, RB[0])
            load_w(nc.scalar, 0, 0, 0, 2)
            load_x(nc.scalar, 0, 1, RB[0])
            load_x(nc.sync, 0, 0, RB[1])
            load_w(nc.scalar, 0, 0, 2, 3)
            load_w(nc.sync, 0, 1, 0, 2)
            load_x(nc.scalar, 0, 1, RB[1])
            load_w(nc.sync, 0, 1, 2, 3)
            load_x(nc.sync, 0, 0, RB[2])
            load_x(nc.scalar, 0, 1, RB[2])
            load_x(nc.scalar, 1, 0, slice(0, ROWS_A))
            load_x(nc.scalar, 1, 0, slice(ROWS_A, H))
            load_x(nc.scalar, 1, 1, slice(0, ROWS_A))
            load_x(nc.scalar, 1, 1, slice(ROWS_A, H))
            load_w(nc.sync, 1, 0, 0, 3)
            load_w(nc.scalar, 1, 1, 0, 3)

            # ---------- ACT func-table warm ----------
            lafs_warm = consts.tile([P, 1], F32)
            nc.scalar.activation(lafs_warm[:], bias_sb[:, 0:1], AF.Identity,
                                 bias=bias_sb[:, 1:2])

            # ---------- PE warm-up: junk matmuls while x streams in ----------
            warm_src = consts.tile([P, WARM_COLS], BF16)
            nc.gpsimd.memset(warm_src[:], 0.0)
            warm_ps = psum.tile([1, WARM_COLS], F32, name="warm_ps", tag="ch0_0")
            for _ in range(N_WARM_MM):
                nc.tensor.matmul(warm_ps[:], warm_src[:, 0:1], warm_src[:],
                                 start=True, stop=True)

            for b in range(B_LOC):
                for it in range(NI):
                    nc.gpsimd.memset(xs[(b, it)][:, 0, :], 0.0)
                    nc.gpsimd.memset(xs[(b, it)][:, HP - 1, :], 0.0)

            # ---------- input transform (DVE): V = B^T d per row band -------
            # tile j covers out cols 2j,2j+1; d0..d3 = x cols 2j-1..2j+2
            # V0=d0-d2  V1=d1+d2  V2=d2-d1  V3=d1-d3
            # edges: j=0 has d0=0 -> V0=-d2 ;  j=31 has d3=0 -> V3=d1
            def make_v(eng, b, it, r):
                x_ = xs[(b, it)]
                v_ = vt[(b, it)]
                rr = slice(r.start + 1, r.stop + 1) if r is not None else slice(0, HP)
                xv = x_[:, rr, :].rearrange("p r (j two) -> p r j two", two=2)
                d1 = xv[:, :, :, 0]                      # cols 0,2,..62   j=0..31
                d2 = xv[:, :, :, 1]                      # cols 1,3,..63   j=0..31
                xo = x_[:, rr, 1:63].rearrange("p r (j two) -> p r j two", two=2)
                d0i = xo[:, :, :, 0]                     # cols 1,3,..61   j=1..31
                d3i = x_[:, rr, 2:64].rearrange(
                    "p r (j two) -> p r j two", two=2)[:, :, :, 0]  # cols 2..62, j=0..30
                sub, add = mybir.AluOpType.subtract, mybir.AluOpType.add
                v_sl = v_[:, rr]
                eng.tensor_tensor(v_sl[:, :, 0, 1:NJ], d0i, d2[:, :, 1:], sub)
                eng.tensor_scalar_mul(v_sl[:, :, 0, 0:1], d2[:, :, 0:1], -1.0)
                eng.tensor_tensor(v_sl[:, :, 1, :], d1, d2, add)
                eng.tensor_tensor(v_sl[:, :, 2, :], d2, d1, sub)
                eng.tensor_tensor(v_sl[:, :, 3, 0:NJ - 1],
                                  d1[:, :, 0:NJ - 1], d3i, sub)
                eng.tensor_copy(v_sl[:, :, 3, NJ - 1:NJ], d1[:, :, NJ - 1:NJ])

            # pad rows: compute V over the pad rows too (xs pads are zero,
            # so V is zero there) by extending the first/last band.  The
            # startup-critical first bands run on the DVE; the rest on the
            # otherwise-idle GPSIMD.
            make_v(nc.vector, 0, 0, slice(-1, 17))
            make_v(nc.vector, 0, 1, slice(-1, 17))
            make_v(nc.vector, 0, 0, RB[1])
            make_v(nc.vector, 0, 1, RB[1])
            _veng = nc.gpsimd if V_REST_GPS else nc.vector
            make_v(_veng, 0, 0, slice(ROWS_A, H + 1))
            make_v(_veng, 0, 1, slice(ROWS_A, H + 1))

            # ---------- main conv block ----------
            # per 8-row chunk and transform channel t: accumulate over
            # (it, dy) into a [P, 8, NJ] psum tile, then the DVE folds
            # t-channels into even/odd output columns of the oh tile
            def mm_block(b, ot, r0, nchunks, ctag0=0):
                pcs = []
                for c in range(nchunks):
                    for t in range(NT):
                        pc = psum.tile([P, 8, NJ], F32,
                                       name=f"pc{b}{ot}{r0}_{c}_{t}",
                                       tag=f"ch{(ctag0 + c) % 2}_{t}")
                        pcs.append(pc)
                        for g, (it, dy) in enumerate(
                                (i, d) for i in range(NI) for d in range(3)):
                            lhsT = w_sb[(b, it)][:, dy, t, ot * P:(ot + 1) * P]
                            rs = r0 + c * 8 + dy
                            nc.tensor.matmul(
                                pc[:], lhsT, vt[(b, it)][:, rs:rs + 8, t, :],
                                start=(g == 0), stop=(g == 5),
                            )
                return pcs

            def out_block(b, ot, r0, pcs, ring=None):
                n = len(pcs) // NT
                oh = out_pool.tile([P, 8 * n, W], F32, name=f"oh{b}{ot}{r0}",
                                   tag="oh")
                sub, add = mybir.AluOpType.subtract, mybir.AluOpType.add
                bi = bias_sb[:, ot:ot + 1]
                for c in range(n):
                    m = pcs[c * NT:(c + 1) * NT]
                    # ACT drains each t-channel (single PSUM read per op);
                    # DVE folds in SBUF only
                    st = out_pool.tile([P, NT, 8, NJ], F32,
                                       name=f"st{b}{ot}{r0}_{c}", tag=f"st{c % 2}")
                    for t in range(NT):
                        nc.scalar.activation(st[:, t], m[t][:], AF.Identity)
                    sq = out_pool.tile([P, 2, 8, NJ], F32,
                                       name=f"sq{b}{ot}{r0}_{c}", tag=f"sq{c % 2}")
                    ov = oh[:, c * 8:(c + 1) * 8, :].rearrange(
                        "p r (j two) -> p r j two", two=2)
                    # out_even = ((m0+m1) + bias) + m2
                    # out_odd  = ((m1-m2) + bias) - m3
                    _qeng = nc.gpsimd if QU_GPS else nc.vector
                    _qeng.tensor_tensor(sq[:, 0], st[:, 0], st[:, 1], add)
                    nc.vector.scalar_tensor_tensor(ov[:, :, :, 0], sq[:, 0], bi,
                                                   st[:, 2], add, add)
                    _qeng.tensor_tensor(sq[:, 1], st[:, 1], st[:, 2], sub)
                    nc.vector.scalar_tensor_tensor(ov[:, :, :, 1], sq[:, 1], bi,
                                                   st[:, 3], add, sub)
                (ring or nc.sync).dma_start(
                    out.ap()[b, ot * P:(ot + 1) * P, r0:r0 + 8 * n, :], oh[:])

            # first two sub-blocks are 16-row so the conv starts as soon as
            # the first 17-row x slice lands
            out_block(0, 0, 0, mm_block(0, 0, 0, 2, ctag0=0))
            out_block(0, 0, 16, mm_block(0, 0, 16, 2, ctag0=2))
            out_block(0, 0, 32, mm_block(0, 0, 32, 4))
            out_block(0, 1, 0, mm_block(0, 1, 0, 4))

            # sample-1 input transforms (GPSIMD), emitted mid-sample-0 so
            # they are ready well before the b=1 blocks
            for it in range(NI):
                make_v(nc.gpsimd if V_REST_GPS else nc.vector, 1, it, slice(-1, H + 1))

            out_block(0, 1, 32, mm_block(0, 1, 32, 4))
            for half in range(2):
                out_block(1, 0, half * 32, mm_block(1, 0, half * 32, 4))
            out_block(1, 1, 0, mm_block(1, 1, 0, 4))
            # final blocks shrink so the drain tail is short, stores split
            # across both rings
            out_block(1, 1, 32, mm_block(1, 1, 32, 2, ctag0=0))
            out_block(1, 1, 48, mm_block(1, 1, 48, 1, ctag0=2), ring=nc.scalar)
            out_block(1, 1, 56, mm_block(1, 1, 56, 1, ctag0=3))
    return nc


_CACHED_NC = None
_PREP_JIT = None


def _get_nc():
    global _CACHED_NC
    if _CACHED_NC is None:
        nc = bacc.Bacc("TRN2", target_bir_lowering=False, debug=False,
                       num_devices=N_CORES)
        build_conv2dmod(nc)
        nc.compile()
        _CACHED_NC = nc
    return _CACHED_NC


def _get_prep():
    """jit'd host-side prep on jax-cpu (multithreaded): x -> bf16 and the
    folded, Winograd-transformed per-sample conv weights
    wu[b,i,dy,t,o] = G @ (w*style*winv) in bf16."""
    global _PREP_JIT
    if _PREP_JIT is None:
        import jax
        import jax.numpy as jnp

        cpu = jax.devices("cpu")[0]

        def _prep(x, y, weights, bias, style_w, style_b):
            style = y @ style_w.T + style_b                       # [B, i]
            w2 = jnp.sum(weights * weights, axis=(2, 3))          # [o, i]
            sigma = (style * style) @ w2.T                        # [B, o]
            winv = 1.0 / jnp.sqrt(sigma + EPS)                    # [B, o]
            wfull = (weights[None] * style[:, None, :, None, None]
                     * winv[:, :, None, None, None])              # [B,o,i,dy,dx]
            w0 = wfull[..., 0]
            w1 = wfull[..., 1]
            w2_ = wfull[..., 2]
            u = jnp.stack([w0, 0.5 * (w0 + w1 + w2_),
                           0.5 * (w0 - w1 + w2_), w2_], axis=-1)  # [B,o,i,dy,t]
            wu = u.transpose(0, 2, 3, 4, 1).astype(jnp.bfloat16)  # [B,i,dy,t,o]
            return x.astype(jnp.bfloat16), wu

        _PREP_JIT = (jax.jit(_prep, device=cpu), cpu)
    return _PREP_JIT


def kernel(x, y, weights, bias, style_w, style_b, _trace=False):
    x = np.asarray(x, dtype=np.float32)
    y = np.asarray(y, dtype=np.float32)
    weights = np.asarray(weights, dtype=np.float32)
    bias = np.asarray(bias, dtype=np.float32)
    style_w = np.asarray(style_w, dtype=np.float32)
    style_b = np.asarray(style_b, dtype=np.float32)

    prep, cpu = _get_prep()
    import jax
    with jax.default_device(cpu):
        xbf, wuj = prep(x, y, weights, bias, style_w, style_b)
        xbf = np.asarray(xbf)
        wun = np.asarray(wuj)
    bias_c = np.ascontiguousarray(bias.reshape(NO, P).T)           # [p, oo]

    nc = _get_nc()
    in_maps = []
    for c in range(N_CORES):
        sl = slice(c * B_LOC, (c + 1) * B_LOC)
        in_maps.append({
            "xbf": np.ascontiguousarray(xbf[sl]),
            "wu": np.ascontiguousarray(wun[sl]),
            "bias_col": bias_c,
        })
    res = bass_utils.run_bass_kernel_spmd(
        nc, in_maps, core_ids=list(range(N_CORES)), trace=_trace
    )
    out = np.concatenate([r["out"] for r in res.results], axis=0)
    if _trace:
        kernel.last_results = res
    return out


# revision 31
# speedup vs baseline: 1.1605x; 1.0057x over previous
"""Modulated Conv2D (StyleGAN2-style) Trainium2 Bass kernel.

Problem shapes (hardcoded):
  x: [16, 256, 64, 64] f32    y: [16, 512] f32
  weights: [256, 256, 3, 3]   bias: [256]
  style_w: [256, 512]         style_b: [256]
  out: [16, 256, 64, 64] f32

Two math transformations, both folded on the host while sharding:

1. Modulation/demodulation folded into the weights (style/winv are ~4 MMACs
   of the 77-GFLOP problem):
     out[b,o] = conv(x[b], w * style[b,i] * winv[b,o])[o] + bias[o]
     winv[b,o] = 1/sqrt(sum_i W2[o,i] * style[b,i]^2 + eps)

2. 1-D Winograd F(2,3) along the image width: each 3x3 conv row-tap becomes
   4 transform channels producing 2 output columns per tile, cutting PE
   columns 1.5x (12 tap-channels x 32 tiles vs 9 taps x 64 cols).  The
   weight transform U = G w (G = [[1,0,0],[.5,.5,.5],[.5,-.5,.5],[0,0,1]])
   is folded into the host weight prep; the input transform V = B^T d
   (V0=d0-d2, V1=d1+d2, V2=d2-d1, V3=d1-d3 over even/odd column pairs) runs
   on the DVE; the output transform (out_even = M0+M1+M2, out_odd =
   M1-M2-M3) runs on the DVE reading PSUM; ScalarE adds bias on a final
   copy; row taps (dy) accumulate in PSUM as before.

The device kernel: x (bf16, host-cast) DMAs straight into zero-row-padded
SBUF tiles (columns unpadded -> fully contiguous line-rate DMA; column
edges handled by special-cased V channels at tile 0/31), PE streams 768
256-col matmuls back-to-back, everything else hides in their shadow.

Sharding: data-parallel over batch, 2 samples per core across 8 cores.
Inputs stream over both HWDGE rings (order = priority); a dozen junk
matmuls up front lift the HAM clock gate before the conv arrives.
"""

import numpy as np
import ml_dtypes

import concourse.bass as bass
import concourse.tile as tile
from concourse import bacc, mybir
from concourse import bass_utils

EPS = 1e-8
P = 128
B_LOC = 2          # samples per core
CIN, COUT = 256, 256
NI, NO = CIN // P, COUT // P   # 2, 2
S = 512
H = W = 64
NT = 4             # Winograd F(2,3) transform channels
NJ = W // 2        # 32 column tiles (2 output columns each)
HP = H + 2         # zero-padded rows (columns are not padded)
N_CORES = 8
ROWS_A = 34        # first-chunk rows of the b=1 x load split
N_WARM_MM = 18     # junk matmuls to lift the HAM clock gate before the conv
WARM_COLS = 512    # columns per junk matmul (paces the warm-up)
V_REST_GPS = False  # non-critical V transform bands on GPSIMD
QU_GPS = False      # fold q/u tensor-tensor ops on GPSIMD

F32 = mybir.dt.float32
BF16 = mybir.dt.bfloat16
AF = mybir.ActivationFunctionType


def build_conv2dmod(nc):
    xbf = nc.dram_tensor("xbf", [B_LOC, CIN, H, W], BF16, kind="ExternalInput")
    # Winograd-transformed folded weights: [b, i, dy, t, o]
    wu = nc.dram_tensor("wu", [B_LOC, CIN, 3, NT, COUT], BF16,
                        kind="ExternalInput")
    bias_col = nc.dram_tensor("bias_col", [P, NO], F32, kind="ExternalInput")
    out = nc.dram_tensor("out", [B_LOC, COUT, H, W], F32, kind="ExternalOutput")

    with tile.TileContext(nc) as tc:
        with (
            tc.tile_pool(name="consts", bufs=1) as consts,
            tc.tile_pool(name="xs_pool", bufs=1) as xs_pool,
            tc.tile_pool(name="v_pool", bufs=1) as v_pool,
            tc.tile_pool(name="out_pool", bufs=2) as out_pool,
            tc.tile_pool(name="psum", bufs=1, space="PSUM") as psum,
        ):
            bias_sb = consts.tile([P, NO], F32)
            nc.sync.dma_start(bias_sb[:], bias_col.ap())

            # x tiles: row-padded only, so both the DMA source and the SBUF
            # destination are fully contiguous per partition (line-rate DMA)
            xs = {}
            vt = {}
            for b in range(B_LOC):
                for it in range(NI):
                    xs[(b, it)] = xs_pool.tile([P, HP, W], BF16,
                                               name=f"xs{b}_{it}", tag=f"xs{b}_{it}")
                    vt[(b, it)] = v_pool.tile([P, HP, NT, NJ], BF16,
                                              name=f"v{b}_{it}", tag=f"v{b}_{it}")

            def load_x(eng, b, it, r):
                eng.dma_start(xs[(b, it)][:, r.start + 1:r.stop + 1, :],
                              xbf.ap()[b, it * P:(it + 1) * P, r, :])

            w_sb = {}
            for b in range(B_LOC):
                for it in range(NI):
                    w_sb[(b, it)] = consts.tile([P, 3, NT, COUT], BF16,
                                                name=f"w{b}_{it}", tag=f"w{b}_{it}")

            def load_w(eng, b, it, d0, d1):
                eng.dma_start(w_sb[(b, it)][:, d0:d1], wf_ap(b, it, d0, d1))

            def wf_ap(b, it, d0, d1):
                return wu.ap()[b, it * P:(it + 1) * P, d0:d1]

            # load order = priority; sample-0 x in three row slices so the
            # first conv sub-block can start as early as possible
            RB = (slice(0, 17), slice(17, ROWS_A), slice(ROWS_A, H))
            load_x(nc.sync, 0, 0, RB[0])
            load_w(nc.scalar, 0, 0, 0, 2)
            load_x(nc.scalar, 0, 1, RB[0])
            load_x(nc.sync, 0, 0, RB[1])
            load_w(nc.scalar, 0, 0, 2, 3)
            load_w(nc.sync, 0, 1, 0, 2)
            load_x(nc.scalar, 0, 1, RB[1])
            load_w(nc.sync, 0, 1, 2, 3)
            load_x(nc.sync, 0, 0, RB[2])
            load_x(nc.scalar, 0, 1, RB[2])
            load_x(nc.scalar, 1, 0, slice(0, ROWS_A))
            load_x(nc.scalar, 1, 0, slice(ROWS_A, H))
            load_x(nc.scalar, 1, 1, slice(0, ROWS_A))
            load_x(nc.scalar, 1, 1, slice(ROWS_A, H))
            load_w(nc.sync, 1, 0, 0, 3)
            load_w(nc.scalar, 1, 1, 0, 3)

            # ---------- ACT func-table warm ----------
            lafs_warm = consts.tile([P, 1], F32)
            nc.scalar.activation(lafs_warm[:], bias_sb[:, 0:1], AF.Identity,
                                 bias=bias_sb[:, 1:2])

            # ---------- PE warm-up: junk matmuls while x streams in ----------
            warm_src = consts.tile([P, WARM_COLS], BF16)
            nc.gpsimd.memset(warm_src[:], 0.0)
            warm_ps = psum.tile([1, WARM_COLS], F32, name="warm_ps", tag="ch0_0")
            for _ in range(N_WARM_MM):
                nc.tensor.matmul(warm_ps[:], warm_src[:, 0:1], warm_src[:],
                                 start=True, stop=True)

            for b in range(B_LOC):
                for it in range(NI):
                    nc.gpsimd.memset(xs[(b, it)][:, 0, :], 0.0)
                    nc.gpsimd.memset(xs[(b, it)][:, HP - 1, :], 0.0)

            # ---------- input transform (DVE): V = B^T d per row band -------
            # tile j covers out cols 2j,2j+1; d0..d3 = x cols 2j-1..2j+2
            # V0=d0-d2  V1=d1+d2  V2=d2-d1  V3=d1-d3
            # edges: j=0 has d0=0 -> V0=-d2 ;  j=31 has d3=0 -> V3=d1
            def make_v(eng, b, it, r):
                x_ = xs[(b, it)]
                v_ = vt[(b, it)]
                rr = slice(r.start + 1, r.stop + 1) if r is not None else slice(0, HP)
                xv = x_[:, rr, :].rearrange("p r (j two) -> p r j two", two=2)
                d1 = xv[:, :, :, 0]                      # cols 0,2,..62   j=0..31
                d2 = xv[:, :, :, 1]                      # cols 1,3,..63   j=0..31
                xo = x_[:, rr, 1:63].rearrange("p r (j two) -> p r j two", two=2)
                d0i = xo[:, :, :, 0]                     # cols 1,3,..61   j=1..31
                d3i = x_[:, rr, 2:64].rearrange(
                    "p r (j two) -> p r j two", two=2)[:, :, :, 0]  # cols 2..62, j=0..30
                sub, add = mybir.AluOpType.subtract, mybir.AluOpType.add
                v_sl = v_[:, rr]
                eng.tensor_tensor(v_sl[:, :, 0, 1:NJ], d0i, d2[:, :, 1:], sub)
                eng.tensor_scalar_mul(v_sl[:, :, 0, 0:1], d2[:, :, 0:1], -1.0)
                eng.tensor_tensor(v_sl[:, :, 1, :], d1, d2, add)
                eng.tensor_tensor(v_sl[:, :, 2, :], d2, d1, sub)
                eng.tensor_tensor(v_sl[:, :, 3, 0:NJ - 1],
                                  d1[:, :, 0:NJ - 1], d3i, sub)
                eng.tensor_copy(v_sl[:, :, 3, NJ - 1:NJ], d1[:, :, NJ - 1:NJ])

            # pad rows: compute V over the pad rows too (xs pads are zero,
            # so V is zero there) by extending the first/last band.  The
            # startup-critical first bands run on the DVE; the rest on the
            # otherwise-idle GPSIMD.
            make_v(nc.vector, 0, 0, slice(-1, 17))
            make_v(nc.vector, 0, 1, slice(-1, 17))
            make_v(nc.vector, 0, 0, RB[1])
            make_v(nc.vector, 0, 1, RB[1])
            _veng = nc.gpsimd if V_REST_GPS else nc.vector
            make_v(_veng, 0, 0, slice(ROWS_A, H + 1))
            make_v(_veng, 0, 1, slice(ROWS_A, H + 1))

            # ---------- main conv block ----------
            # per 8-row chunk and transform channel t: accumulate over
            # (it, dy) into a [P, 8, NJ] psum tile, then the DVE folds
            # t-channels into even/odd output columns of the oh tile
            def mm_block(b, ot, r0, nchunks, ctag0=0):
                pcs = []
                for c in range(nchunks):
                    for t in range(NT):
                        pc = psum.tile([P, 8, NJ], F32,
                                       name=f"pc{b}{ot}{r0}_{c}_{t}",
                                       tag=f"ch{(ctag0 + c) % 2}_{t}")
                        pcs.append(pc)
                        for g, (it, dy) in enumerate(
                                (i, d) for i in range(NI) for d in range(3)):
                            lhsT = w_sb[(b, it)][:, dy, t, ot * P:(ot + 1) * P]
                            rs = r0 + c * 8 + dy
                            nc.tensor.matmul(
                                pc[:], lhsT, vt[(b, it)][:, rs:rs + 8, t, :],
                                start=(g == 0), stop=(g == 5),
                            )
                return pcs

            def out_block(b, ot, r0, pcs, ring=None):
                n = len(pcs) // NT
                oh = out_pool.tile([P, 8 * n, W], F32, name=f"oh{b}{ot}{r0}",
                                   tag="oh")
                sub, add = mybir.AluOpType.subtract, mybir.AluOpType.add
                bi = bias_sb[:, ot:ot + 1]
                # ACT drains each t-channel (single PSUM read per op);
                # DVE folds in SBUF only, batched over chunk pairs to
                # amortize per-op overhead
                cp = 0
                while cp < n:
                    nb = min(2, n - cp)
                    m = pcs[cp * NT:(cp + nb) * NT]
                    st = out_pool.tile([P, 2, NT, 8, NJ], F32,
                                       name=f"st{b}{ot}{r0}_{cp}",
                                       tag=f"st{(cp // 2) % 2}")
                    for cc in range(nb):
                        for t in range(NT):
                            nc.scalar.activation(st[:, cc, t], m[cc * NT + t][:],
                                                 AF.Identity)
                    sq = out_pool.tile([P, 2, 2, 8, NJ], F32,
                                       name=f"sq{b}{ot}{r0}_{cp}",
                                       tag=f"sq{(cp // 2) % 2}")
                    stv = st[:, 0:nb]
                    sqv = sq[:, 0:nb]
                    ov = oh[:, cp * 8:(cp + nb) * 8, :].rearrange(
                        "p (c r) (j two) -> p c r j two", c=nb, two=2)
                    # out_even = ((m0+m1) + bias) + m2
                    # out_odd  = ((m1-m2) + bias) - m3
                    nc.vector.tensor_tensor(sqv[:, :, 0], stv[:, :, 0],
                                            stv[:, :, 1], add)
                    nc.vector.scalar_tensor_tensor(ov[:, :, :, :, 0],
                                                   sqv[:, :, 0], bi,
                                                   stv[:, :, 2], add, add)
                    nc.vector.tensor_tensor(sqv[:, :, 1], stv[:, :, 1],
                                            stv[:, :, 2], sub)
                    nc.vector.scalar_tensor_tensor(ov[:, :, :, :, 1],
                                                   sqv[:, :, 1], bi,
                                                   stv[:, :, 3], add, sub)
                    cp += nb
                (ring or nc.sync).dma_start(
                    out.ap()[b, ot * P:(ot + 1) * P, r0:r0 + 8 * n, :], oh[:])

            # first two sub-blocks are 16-row so the conv starts as soon as
            # the first 17-row x slice lands
            out_block(0, 0, 0, mm_block(0, 0, 0, 2, ctag0=0))
            out_block(0, 0, 16, mm_block(0, 0, 16, 2, ctag0=2))
            out_block(0, 0, 32, mm_block(0, 0, 32, 4))
            out_block(0, 1, 0, mm_block(0, 1, 0, 4))

            # sample-1 input transforms (GPSIMD), emitted mid-sample-0 so
            # they are ready well before the b=1 blocks
            for it in range(NI):
                make_v(nc.gpsimd if V_REST_GPS else nc.vector, 1, it, slice(-1, H + 1))

            out_block(0, 1, 32, mm_block(0, 1, 32, 4))
            for half in range(2):
                out_block(1, 0, half * 32, mm_block(1, 0, half * 32, 4))
            out_block(1, 1, 0, mm_block(1, 1, 0, 4))
            # final blocks shrink so the drain tail is short, stores split
            # across both rings
            out_block(1, 1, 32, mm_block(1, 1, 32, 2, ctag0=0))
            out_block(1, 1, 48, mm_block(1, 1, 48, 1, ctag0=2), ring=nc.scalar)
            out_block(1, 1, 56, mm_block(1, 1, 56, 1, ctag0=3))
    return nc


_CACHED_NC = None
_PREP_JIT = None


def _get_nc():
    global _CACHED_NC
    if _CACHED_NC is None:
        nc = bacc.Bacc("TRN2", target_bir_lowering=False, debug=False,
                       num_devices=N_CORES)
        build_conv2dmod(nc)
        nc.compile()
        _CACHED_NC = nc
    return _CACHED_NC


def _get_prep():
    """jit'd host-side prep on jax-cpu (multithreaded): x -> bf16 and the
    folded, Winograd-transformed per-sample conv weights
    wu[b,i,dy,t,o] = G @ (w*style*winv) in bf16."""
    global _PREP_JIT
    if _PREP_JIT is None:
        import jax
        import jax.numpy as jnp

        cpu = jax.devices("cpu")[0]

        def _prep(x, y, weights, bias, style_w, style_b):
            style = y @ style_w.T + style_b                       # [B, i]
            w2 = jnp.sum(weights * weights, axis=(2, 3))          # [o, i]
            sigma = (style * style) @ w2.T                        # [B, o]
            winv = 1.0 / jnp.sqrt(sigma + EPS)                    # [B, o]
            wfull = (weights[None] * style[:, None, :, None, None]
                     * winv[:, :, None, None, None])              # [B,o,i,dy,dx]
            w0 = wfull[..., 0]
            w1 = wfull[..., 1]
            w2_ = wfull[..., 2]
            u = jnp.stack([w0, 0.5 * (w0 + w1 + w2_),
                           0.5 * (w0 - w1 + w2_), w2_], axis=-1)  # [B,o,i,dy,t]
            wu = u.transpose(0, 2, 3, 4, 1).astype(jnp.bfloat16)  # [B,i,dy,t,o]
            return x.astype(jnp.bfloat16), wu

        _PREP_JIT = (jax.jit(_prep, device=cpu), cpu)
    return _PREP_JIT


def kernel(x, y, weights, bias, style_w, style_b, _trace=False):
    x = np.asarray(x, dtype=np.float32)
    y = np.asarray(y, dtype=np.float32)
    weights = np.asarray(weights, dtype=np.float32)
    bias = np.asarray(bias, dtype=np.float32)
    style_w = np.asarray(style_w, dtype=np.float32)
    style_b = np.asarray(style_b, dtype=np.float32)

    prep, cpu = _get_prep()
    import jax
    with jax.default_device(cpu):
        xbf, wuj = prep(x, y, weights, bias, style_w, style_b)
        xbf = np.asarray(xbf)
        wun = np.asarray(wuj)
    bias_c = np.ascontiguousarray(bias.reshape(NO, P).T)           # [p, oo]

    nc = _get_nc()
    in_maps = []
    for c in range(N_CORES):
        sl = slice(c * B_LOC, (c + 1) * B_LOC)
        in_maps.append({
            "xbf": np.ascontiguousarray(xbf[sl]),
            "wu": np.ascontiguousarray(wun[sl]),
            "bias_col": bias_c,
        })
    res = bass_utils.run_bass_kernel_spmd(
        nc, in_maps, core_ids=list(range(N_CORES)), trace=_trace
    )
    out = np.concatenate([r["out"] for r in res.results], axis=0)
    if _trace:
        kernel.last_results = res
    return out


# revision 32
# speedup vs baseline: 1.1703x; 1.0085x over previous
"""Modulated Conv2D (StyleGAN2-style) Trainium2 Bass kernel.

Problem shapes (hardcoded):
  x: [16, 256, 64, 64] f32    y: [16, 512] f32
  weights: [256, 256, 3, 3]   bias: [256]
  style_w: [256, 512]         style_b: [256]
  out: [16, 256, 64, 64] f32

Two math transformations, both folded on the host while sharding:

1. Modulation/demodulation folded into the weights (style/winv are ~4 MMACs
   of the 77-GFLOP problem):
     out[b,o] = conv(x[b], w * style[b,i] * winv[b,o])[o] + bias[o]
     winv[b,o] = 1/sqrt(sum_i W2[o,i] * style[b,i]^2 + eps)

2. 1-D Winograd F(2,3) along the image width: each 3x3 conv row-tap becomes
   4 transform channels producing 2 output columns per tile, cutting PE
   columns 1.5x (12 tap-channels x 32 tiles vs 9 taps x 64 cols).  The
   weight transform U = G w (G = [[1,0,0],[.5,.5,.5],[.5,-.5,.5],[0,0,1]])
   is folded into the host weight prep; the input transform V = B^T d
   (V0=d0-d2, V1=d1+d2, V2=d2-d1, V3=d1-d3 over even/odd column pairs) runs
   on the DVE; the output transform (out_even = M0+M1+M2, out_odd =
   M1-M2-M3) runs on the DVE reading PSUM; ScalarE adds bias on a final
   copy; row taps (dy) accumulate in PSUM as before.

The device kernel: x (bf16, host-cast) DMAs straight into zero-row-padded
SBUF tiles (columns unpadded -> fully contiguous line-rate DMA; column
edges handled by special-cased V channels at tile 0/31), PE streams 768
256-col matmuls back-to-back, everything else hides in their shadow.

Sharding: data-parallel over batch, 2 samples per core across 8 cores.
Inputs stream over both HWDGE rings (order = priority); a dozen junk
matmuls up front lift the HAM clock gate before the conv arrives.
"""

import numpy as np

import concourse.bass as bass
import concourse.tile as tile
from concourse import bacc, mybir
from concourse import bass_utils

EPS = 1e-8
P = 128
B_LOC = 2          # samples per core
CIN, COUT = 256, 256
NI, NO = CIN // P, COUT // P   # 2, 2
S = 512
H = W = 64
NT = 4             # Winograd F(2,3) transform channels
NJ = W // 2        # 32 column tiles (2 output columns each)
HP = H + 2         # zero-padded rows (columns are not padded)
N_CORES = 8
ROWS_A = 34        # first-chunk rows of the b=1 x load split
N_WARM_MM = 18     # junk matmuls to lift the HAM clock gate before the conv
WARM_COLS = 512    # columns per junk matmul (paces the warm-up)

F32 = mybir.dt.float32
BF16 = mybir.dt.bfloat16
AF = mybir.ActivationFunctionType


def build_conv2dmod(nc):
    xbf = nc.dram_tensor("xbf", [B_LOC, CIN, H, W], BF16, kind="ExternalInput")
    # Winograd-transformed folded weights: [b, i, dy, t, o]
    wu = nc.dram_tensor("wu", [B_LOC, CIN, 3, NT, COUT], BF16,
                        kind="ExternalInput")
    bias_col = nc.dram_tensor("bias_col", [P, NO], F32, kind="ExternalInput")
    out = nc.dram_tensor("out", [B_LOC, COUT, H, W], F32, kind="ExternalOutput")

    with tile.TileContext(nc) as tc:
        with (
            tc.tile_pool(name="consts", bufs=1) as consts,
            tc.tile_pool(name="xs_pool", bufs=1) as xs_pool,
            tc.tile_pool(name="v_pool", bufs=1) as v_pool,
            tc.tile_pool(name="out_pool", bufs=2) as out_pool,
            tc.tile_pool(name="psum", bufs=1, space="PSUM") as psum,
        ):
            bias_sb = consts.tile([P, NO], F32)
            nc.sync.dma_start(bias_sb[:], bias_col.ap())

            # x tiles: row-padded only, so both the DMA source and the SBUF
            # destination are fully contiguous per partition (line-rate DMA)
            xs = {}
            vt = {}
            for b in range(B_LOC):
                for it in range(NI):
                    xs[(b, it)] = xs_pool.tile([P, HP, W], BF16,
                                               name=f"xs{b}_{it}", tag=f"xs{b}_{it}")
                    vt[(b, it)] = v_pool.tile([P, HP, NT, NJ], BF16,
                                              name=f"v{b}_{it}", tag=f"v{b}_{it}")

            def load_x(eng, b, it, r):
                eng.dma_start(xs[(b, it)][:, r.start + 1:r.stop + 1, :],
                              xbf.ap()[b, it * P:(it + 1) * P, r, :])

            w_sb = {}
            for b in range(B_LOC):
                for it in range(NI):
                    w_sb[(b, it)] = consts.tile([P, 3, NT, COUT], BF16,
                                                name=f"w{b}_{it}", tag=f"w{b}_{it}")

            def load_w(eng, b, it, d0, d1):
                eng.dma_start(w_sb[(b, it)][:, d0:d1], wf_ap(b, it, d0, d1))

            def wf_ap(b, it, d0, d1):
                return wu.ap()[b, it * P:(it + 1) * P, d0:d1]

            # load order = priority; sample-0 x in three row slices so the
            # first conv sub-block can start as early as possible
            RB = (slice(0, 17), slice(17, ROWS_A), slice(ROWS_A, H))
            load_x(nc.sync, 0, 0, RB[0])
            load_w(nc.scalar, 0, 0, 0, 2)
            load_x(nc.scalar, 0, 1, RB[0])
            load_x(nc.sync, 0, 0, RB[1])
            load_w(nc.scalar, 0, 0, 2, 3)
            load_w(nc.sync, 0, 1, 0, 2)
            load_x(nc.scalar, 0, 1, RB[1])
            load_w(nc.sync, 0, 1, 2, 3)
            load_x(nc.sync, 0, 0, RB[2])
            load_x(nc.scalar, 0, 1, RB[2])
            load_x(nc.scalar, 1, 0, slice(0, ROWS_A))
            load_x(nc.scalar, 1, 0, slice(ROWS_A, H))
            load_x(nc.scalar, 1, 1, slice(0, ROWS_A))
            load_x(nc.scalar, 1, 1, slice(ROWS_A, H))
            load_w(nc.sync, 1, 0, 0, 3)
            load_w(nc.scalar, 1, 1, 0, 3)

            # ---------- ACT func-table warm ----------
            lafs_warm = consts.tile([P, 1], F32)
            nc.scalar.activation(lafs_warm[:], bias_sb[:, 0:1], AF.Identity,
                                 bias=bias_sb[:, 1:2])

            # ---------- PE warm-up: junk matmuls while x streams in ----------
            warm_src = consts.tile([P, WARM_COLS], BF16)
            nc.gpsimd.memset(warm_src[:], 0.0)
            warm_ps = psum.tile([1, WARM_COLS], F32, name="warm_ps", tag="ch0_0")
            for _ in range(N_WARM_MM):
                nc.tensor.matmul(warm_ps[:], warm_src[:, 0:1], warm_src[:],
                                 start=True, stop=True)

            for b in range(B_LOC):
                for it in range(NI):
                    nc.gpsimd.memset(xs[(b, it)][:, 0, :], 0.0)
                    nc.gpsimd.memset(xs[(b, it)][:, HP - 1, :], 0.0)

            # ---------- input transform (DVE): V = B^T d per row band -------
            # tile j covers out cols 2j,2j+1; d0..d3 = x cols 2j-1..2j+2
            # V0=d0-d2  V1=d1+d2  V2=d2-d1  V3=d1-d3
            # edges: j=0 has d0=0 -> V0=-d2 ;  j=31 has d3=0 -> V3=d1
            def make_v(eng, b, it, r):
                x_ = xs[(b, it)]
                v_ = vt[(b, it)]
                rr = slice(r.start + 1, r.stop + 1) if r is not None else slice(0, HP)
                xv = x_[:, rr, :].rearrange("p r (j two) -> p r j two", two=2)
                d1 = xv[:, :, :, 0]                      # cols 0,2,..62   j=0..31
                d2 = xv[:, :, :, 1]                      # cols 1,3,..63   j=0..31
                xo = x_[:, rr, 1:63].rearrange("p r (j two) -> p r j two", two=2)
                d0i = xo[:, :, :, 0]                     # cols 1,3,..61   j=1..31
                d3i = x_[:, rr, 2:64].rearrange(
                    "p r (j two) -> p r j two", two=2)[:, :, :, 0]  # cols 2..62, j=0..30
                sub, add = mybir.AluOpType.subtract, mybir.AluOpType.add
                v_sl = v_[:, rr]
                eng.tensor_tensor(v_sl[:, :, 0, 1:NJ], d0i, d2[:, :, 1:], sub)
                eng.tensor_scalar_mul(v_sl[:, :, 0, 0:1], d2[:, :, 0:1], -1.0)
                eng.tensor_tensor(v_sl[:, :, 1, :], d1, d2, add)
                eng.tensor_tensor(v_sl[:, :, 2, :], d2, d1, sub)
                eng.tensor_tensor(v_sl[:, :, 3, 0:NJ - 1],
                                  d1[:, :, 0:NJ - 1], d3i, sub)
                eng.tensor_copy(v_sl[:, :, 3, NJ - 1:NJ], d1[:, :, NJ - 1:NJ])

            # pad rows: compute V over the pad rows too (xs pads are zero,
            # so V is zero there) by extending the first/last band.  The
            # startup-critical first bands run on the DVE; the rest on the
            # otherwise-idle GPSIMD.
            make_v(nc.vector, 0, 0, slice(-1, 17))
            make_v(nc.vector, 0, 1, slice(-1, 17))
            make_v(nc.vector, 0, 0, RB[1])
            make_v(nc.vector, 0, 1, RB[1])
            _veng = nc.vector
            make_v(_veng, 0, 0, slice(ROWS_A, H + 1))
            make_v(_veng, 0, 1, slice(ROWS_A, H + 1))

            # ---------- main conv block ----------
            # per 8-row chunk and transform channel t: accumulate over
            # (it, dy) into a [P, 8, NJ] psum tile, then the DVE folds
            # t-channels into even/odd output columns of the oh tile
            def mm_block(b, ot, r0, nchunks, ctag0=0):
                pcs = []
                for c in range(nchunks):
                    for t in range(NT):
                        pc = psum.tile([P, 8, NJ], F32,
                                       name=f"pc{b}{ot}{r0}_{c}_{t}",
                                       tag=f"ch{(ctag0 + c) % 2}_{t}")
                        pcs.append(pc)
                        for g, (it, dy) in enumerate(
                                (i, d) for i in range(NI) for d in range(3)):
                            lhsT = w_sb[(b, it)][:, dy, t, ot * P:(ot + 1) * P]
                            rs = r0 + c * 8 + dy
                            nc.tensor.matmul(
                                pc[:], lhsT, vt[(b, it)][:, rs:rs + 8, t, :],
                                start=(g == 0), stop=(g == 5),
                            )
                return pcs

            def out_block(b, ot, r0, pcs, ring=None):
                n = len(pcs) // NT
                oh = out_pool.tile([P, 8 * n, W], F32, name=f"oh{b}{ot}{r0}",
                                   tag="oh")
                sub, add = mybir.AluOpType.subtract, mybir.AluOpType.add
                bi = bias_sb[:, ot:ot + 1]
                # ACT drains each t-channel (single PSUM read per op);
                # DVE folds in SBUF only, batched over chunk pairs to
                # amortize per-op overhead
                cp = 0
                while cp < n:
                    nb = min(2, n - cp)
                    m = pcs[cp * NT:(cp + nb) * NT]
                    st = out_pool.tile([P, 2, NT, 8, NJ], F32,
                                       name=f"st{b}{ot}{r0}_{cp}",
                                       tag=f"st{(cp // 2) % 2}")
                    for cc in range(nb):
                        for t in range(NT):
                            nc.scalar.activation(st[:, cc, t], m[cc * NT + t][:],
                                                 AF.Identity)
                    sq = out_pool.tile([P, 2, 2, 8, NJ], F32,
                                       name=f"sq{b}{ot}{r0}_{cp}",
                                       tag=f"sq{(cp // 2) % 2}")
                    stv = st[:, 0:nb]
                    sqv = sq[:, 0:nb]
                    ov = oh[:, cp * 8:(cp + nb) * 8, :].rearrange(
                        "p (c r) (j two) -> p c r j two", c=nb, two=2)
                    # out_even = ((m0+m1) + bias) + m2
                    # out_odd  = ((m1-m2) + bias) - m3
                    nc.vector.tensor_tensor(sqv[:, :, 0], stv[:, :, 0],
                                            stv[:, :, 1], add)
                    nc.vector.scalar_tensor_tensor(ov[:, :, :, :, 0],
                                                   sqv[:, :, 0], bi,
                                                   stv[:, :, 2], add, add)
                    nc.vector.tensor_tensor(sqv[:, :, 1], stv[:, :, 1],
                                            stv[:, :, 2], sub)
                    nc.vector.scalar_tensor_tensor(ov[:, :, :, :, 1],
                                                   sqv[:, :, 1], bi,
                                                   stv[:, :, 3], add, sub)
                    cp += nb
                (ring or nc.sync).dma_start(
                    out.ap()[b, ot * P:(ot + 1) * P, r0:r0 + 8 * n, :], oh[:])

            # first two sub-blocks are 16-row so the conv starts as soon as
            # the first 17-row x slice lands
            out_block(0, 0, 0, mm_block(0, 0, 0, 2, ctag0=0))
            out_block(0, 0, 16, mm_block(0, 0, 16, 2, ctag0=2))
            out_block(0, 0, 32, mm_block(0, 0, 32, 4))
            out_block(0, 1, 0, mm_block(0, 1, 0, 4))

            # sample-1 input transforms (GPSIMD), emitted mid-sample-0 so
            # they are ready well before the b=1 blocks
            for it in range(NI):
                make_v(nc.vector, 1, it, slice(-1, H + 1))

            out_block(0, 1, 32, mm_block(0, 1, 32, 4))
            for half in range(2):
                out_block(1, 0, half * 32, mm_block(1, 0, half * 32, 4))
            out_block(1, 1, 0, mm_block(1, 1, 0, 4))
            # final blocks shrink so the drain tail is short, stores split
            # across both rings
            out_block(1, 1, 32, mm_block(1, 1, 32, 2, ctag0=0))
            out_block(1, 1, 48, mm_block(1, 1, 48, 1, ctag0=2), ring=nc.scalar)
            out_block(1, 1, 56, mm_block(1, 1, 56, 1, ctag0=3))
    return nc


_CACHED_NC = None
_PREP_JIT = None


def _get_nc():
    global _CACHED_NC
    if _CACHED_NC is None:
        nc = bacc.Bacc("TRN2", target_bir_lowering=False, debug=False,
                       num_devices=N_CORES)
        build_conv2dmod(nc)
        nc.compile()
        _CACHED_NC = nc
    return _CACHED_NC


def _get_prep():
    """jit'd host-side prep on jax-cpu (multithreaded): x -> bf16 and the
    folded, Winograd-transformed per-sample conv weights
    wu[b,i,dy,t,o] = G @ (w*style*winv) in bf16."""
    global _PREP_JIT
    if _PREP_JIT is None:
        import jax
        import jax.numpy as jnp

        cpu = jax.devices("cpu")[0]

        def _prep(x, y, weights, bias, style_w, style_b):
            style = y @ style_w.T + style_b                       # [B, i]
            w2 = jnp.sum(weights * weights, axis=(2, 3))          # [o, i]
            sigma = (style * style) @ w2.T                        # [B, o]
            winv = 1.0 / jnp.sqrt(sigma + EPS)                    # [B, o]
            wfull = (weights[None] * style[:, None, :, None, None]
                     * winv[:, :, None, None, None])              # [B,o,i,dy,dx]
            w0 = wfull[..., 0]
            w1 = wfull[..., 1]
            w2_ = wfull[..., 2]
            u = jnp.stack([w0, 0.5 * (w0 + w1 + w2_),
                           0.5 * (w0 - w1 + w2_), w2_], axis=-1)  # [B,o,i,dy,t]
            wu = u.transpose(0, 2, 3, 4, 1).astype(jnp.bfloat16)  # [B,i,dy,t,o]
            return x.astype(jnp.bfloat16), wu

        _PREP_JIT = (jax.jit(_prep, device=cpu), cpu)
    return _PREP_JIT


def kernel(x, y, weights, bias, style_w, style_b, _trace=False):
    x = np.asarray(x, dtype=np.float32)
    y = np.asarray(y, dtype=np.float32)
    weights = np.asarray(weights, dtype=np.float32)
    bias = np.asarray(bias, dtype=np.float32)
    style_w = np.asarray(style_w, dtype=np.float32)
    style_b = np.asarray(style_b, dtype=np.float32)

    prep, cpu = _get_prep()
    import jax
    with jax.default_device(cpu):
        xbf, wuj = prep(x, y, weights, bias, style_w, style_b)
        xbf = np.asarray(xbf)
        wun = np.asarray(wuj)
    bias_c = np.ascontiguousarray(bias.reshape(NO, P).T)           # [p, oo]

    nc = _get_nc()
    in_maps = []
    for c in range(N_CORES):
        sl = slice(c * B_LOC, (c + 1) * B_LOC)
        in_maps.append({
            "xbf": np.ascontiguousarray(xbf[sl]),
            "wu": np.ascontiguousarray(wun[sl]),
            "bias_col": bias_c,
        })
    res = bass_utils.run_bass_kernel_spmd(
        nc, in_maps, core_ids=list(range(N_CORES)), trace=_trace
    )
    out = np.concatenate([r["out"] for r in res.results], axis=0)
    if _trace:
        kernel.last_results = res
    return out


# revision 33
# speedup vs baseline: 1.1878x; 1.0150x over previous
"""Modulated Conv2D (StyleGAN2-style) Trainium2 Bass kernel.

Problem shapes (hardcoded):
  x: [16, 256, 64, 64] f32    y: [16, 512] f32
  weights: [256, 256, 3, 3]   bias: [256]
  style_w: [256, 512]         style_b: [256]
  out: [16, 256, 64, 64] f32

Two math transformations, both folded on the host while sharding:

1. Modulation/demodulation folded into the weights (style/winv are ~4 MMACs
   of the 77-GFLOP problem):
     out[b,o] = conv(x[b], w * style[b,i] * winv[b,o])[o] + bias[o]
     winv[b,o] = 1/sqrt(sum_i W2[o,i] * style[b,i]^2 + eps)

2. 1-D Winograd F(2,3) along the image width: each 3x3 conv row-tap becomes
   4 transform channels producing 2 output columns per tile, cutting PE
   columns 1.5x (12 tap-channels x 32 tiles vs 9 taps x 64 cols).  The
   weight transform U = G w (G = [[1,0,0],[.5,.5,.5],[.5,-.5,.5],[0,0,1]])
   is folded into the host weight prep; the input transform V = B^T d
   (V0=d0-d2, V1=d1+d2, V2=d2-d1, V3=d1-d3 over even/odd column pairs) runs
   on the DVE; the output transform (out_even = M0+M1+M2, out_odd =
   M1-M2-M3) runs on the DVE reading PSUM; ScalarE adds bias on a final
   copy; row taps (dy) accumulate in PSUM as before.

The device kernel: x (bf16, host-cast) DMAs straight into zero-row-padded
SBUF tiles (columns unpadded -> fully contiguous line-rate DMA; column
edges handled by special-cased V channels at tile 0/31), PE streams 768
256-col matmuls back-to-back, everything else hides in their shadow.

Sharding: data-parallel over batch, 2 samples per core across 8 cores.
Inputs stream over both HWDGE rings (order = priority); a dozen junk
matmuls up front lift the HAM clock gate before the conv arrives.
"""

import numpy as np

import concourse.bass as bass
import concourse.tile as tile
from concourse import bacc, mybir
from concourse import bass_utils

EPS = 1e-8
P = 128
B_LOC = 2          # samples per core
CIN, COUT = 256, 256
NI, NO = CIN // P, COUT // P   # 2, 2
S = 512
H = W = 64
NT = 4             # Winograd F(2,3) transform channels
NJ = W // 2        # 32 column tiles (2 output columns each)
HP = H + 2         # zero-padded rows (columns are not padded)
N_CORES = 8
ROWS_A = 34        # first-chunk rows of the b=1 x load split
N_WARM_MM = 18     # junk matmuls to lift the HAM clock gate before the conv
WARM_COLS = 512    # columns per junk matmul (paces the warm-up)

F32 = mybir.dt.float32
BF16 = mybir.dt.bfloat16
AF = mybir.ActivationFunctionType


def build_conv2dmod(nc):
    xbf = nc.dram_tensor("xbf", [B_LOC, CIN, H, W], BF16, kind="ExternalInput")
    # Winograd-transformed folded weights: [b, i, dy, t, o]
    wu = nc.dram_tensor("wu", [B_LOC, CIN, 3, NT, COUT], BF16,
                        kind="ExternalInput")
    bias_col = nc.dram_tensor("bias_col", [P, NO], F32, kind="ExternalInput")
    out = nc.dram_tensor("out", [B_LOC, COUT, H, W], F32, kind="ExternalOutput")

    with tile.TileContext(nc) as tc:
        with (
            tc.tile_pool(name="consts", bufs=1) as consts,
            tc.tile_pool(name="xs_pool", bufs=1) as xs_pool,
            tc.tile_pool(name="v_pool", bufs=1) as v_pool,
            tc.tile_pool(name="out_pool", bufs=2) as out_pool,
            tc.tile_pool(name="psum", bufs=1, space="PSUM") as psum,
        ):
            bias_sb = consts.tile([P, NO], F32)
            nc.sync.dma_start(bias_sb[:], bias_col.ap())

            # x tiles: row-padded only, so both the DMA source and the SBUF
            # destination are fully contiguous per partition (line-rate DMA)
            xs = {}
            vt = {}
            for b in range(B_LOC):
                for it in range(NI):
                    xs[(b, it)] = xs_pool.tile([P, HP, W], BF16,
                                               name=f"xs{b}_{it}", tag=f"xs{b}_{it}")
                    vt[(b, it)] = v_pool.tile([P, HP, NT, NJ], BF16,
                                              name=f"v{b}_{it}", tag=f"v{b}_{it}")

            def load_x(eng, b, it, r):
                eng.dma_start(xs[(b, it)][:, r.start + 1:r.stop + 1, :],
                              xbf.ap()[b, it * P:(it + 1) * P, r, :])

            w_sb = {}
            for b in range(B_LOC):
                for it in range(NI):
                    w_sb[(b, it)] = consts.tile([P, 3, NT, COUT], BF16,
                                                name=f"w{b}_{it}", tag=f"w{b}_{it}")

            def load_w(eng, b, it, d0, d1):
                eng.dma_start(w_sb[(b, it)][:, d0:d1], wf_ap(b, it, d0, d1))

            def wf_ap(b, it, d0, d1):
                return wu.ap()[b, it * P:(it + 1) * P, d0:d1]

            # load order = priority; sample-0 x in three row slices so the
            # first conv sub-block can start as early as possible
            RB = (slice(0, 17), slice(17, ROWS_A), slice(ROWS_A, H))
            load_x(nc.sync, 0, 0, RB[0])
            load_w(nc.scalar, 0, 0, 0, 2)
            load_x(nc.scalar, 0, 1, RB[0])
            load_x(nc.sync, 0, 0, RB[1])
            load_w(nc.scalar, 0, 0, 2, 3)
            load_w(nc.sync, 0, 1, 0, 2)
            load_x(nc.scalar, 0, 1, RB[1])
            load_w(nc.sync, 0, 1, 2, 3)
            load_x(nc.sync, 0, 0, RB[2])
            load_x(nc.scalar, 0, 1, RB[2])
            load_x(nc.scalar, 1, 0, slice(0, ROWS_A))
            load_x(nc.scalar, 1, 0, slice(ROWS_A, H))
            load_x(nc.scalar, 1, 1, slice(0, ROWS_A))
            load_x(nc.scalar, 1, 1, slice(ROWS_A, H))
            load_w(nc.sync, 1, 0, 0, 3)
            load_w(nc.scalar, 1, 1, 0, 3)

            # ---------- ACT func-table warm ----------
            lafs_warm = consts.tile([P, 1], F32)
            nc.scalar.activation(lafs_warm[:], bias_sb[:, 0:1], AF.Identity,
                                 bias=bias_sb[:, 1:2])

            # ---------- PE warm-up: junk matmuls while x streams in ----------
            warm_src = consts.tile([P, WARM_COLS], BF16)
            nc.gpsimd.memset(warm_src[:], 0.0)
            warm_ps = psum.tile([1, WARM_COLS], F32, name="warm_ps", tag="ch0_0")
            for _ in range(N_WARM_MM):
                nc.tensor.matmul(warm_ps[:], warm_src[:, 0:1], warm_src[:],
                                 start=True, stop=True)

            for b in range(B_LOC):
                for it in range(NI):
                    nc.gpsimd.memset(xs[(b, it)][:, 0, :], 0.0)
                    nc.gpsimd.memset(xs[(b, it)][:, HP - 1, :], 0.0)

            # ---------- input transform (DVE): V = B^T d per row band -------
            # tile j covers out cols 2j,2j+1; d0..d3 = x cols 2j-1..2j+2
            # V0=d0-d2  V1=d1+d2  V2=d2-d1  V3=d1-d3
            # edges: j=0 has d0=0 -> V0=-d2 ;  j=31 has d3=0 -> V3=d1
            def make_v(eng, b, it, r):
                x_ = xs[(b, it)]
                v_ = vt[(b, it)]
                rr = slice(r.start + 1, r.stop + 1) if r is not None else slice(0, HP)
                xv = x_[:, rr, :].rearrange("p r (j two) -> p r j two", two=2)
                d1 = xv[:, :, :, 0]                      # cols 0,2,..62   j=0..31
                d2 = xv[:, :, :, 1]                      # cols 1,3,..63   j=0..31
                xo = x_[:, rr, 1:63].rearrange("p r (j two) -> p r j two", two=2)
                d0i = xo[:, :, :, 0]                     # cols 1,3,..61   j=1..31
                d3i = x_[:, rr, 2:64].rearrange(
                    "p r (j two) -> p r j two", two=2)[:, :, :, 0]  # cols 2..62, j=0..30
                sub, add = mybir.AluOpType.subtract, mybir.AluOpType.add
                v_sl = v_[:, rr]
                eng.tensor_tensor(v_sl[:, :, 0, 1:NJ], d0i, d2[:, :, 1:], sub)
                eng.tensor_scalar_mul(v_sl[:, :, 0, 0:1], d2[:, :, 0:1], -1.0)
                eng.tensor_tensor(v_sl[:, :, 1, :], d1, d2, add)
                eng.tensor_tensor(v_sl[:, :, 2, :], d2, d1, sub)
                eng.tensor_tensor(v_sl[:, :, 3, 0:NJ - 1],
                                  d1[:, :, 0:NJ - 1], d3i, sub)
                eng.tensor_copy(v_sl[:, :, 3, NJ - 1:NJ], d1[:, :, NJ - 1:NJ])

            # pad rows: compute V over the pad rows too (xs pads are zero,
            # so V is zero there) by extending the first/last band.  The
            # startup-critical first bands run on the DVE; the rest on the
            # otherwise-idle GPSIMD.
            make_v(nc.vector, 0, 0, slice(-1, 17))
            make_v(nc.vector, 0, 1, slice(-1, 17))
            make_v(nc.vector, 0, 0, RB[1])
            make_v(nc.vector, 0, 1, RB[1])
            _veng = nc.vector
            make_v(_veng, 0, 0, slice(ROWS_A, H + 1))
            make_v(_veng, 0, 1, slice(ROWS_A, H + 1))

            # ---------- main conv block ----------
            # per 8-row chunk and transform channel t: accumulate over
            # (it, dy) into a [P, 8, NJ] psum tile, then the DVE folds
            # t-channels into even/odd output columns of the oh tile
            def mm_block(b, ot, r0, nchunks, ctag0=0):
                # 16-row chunks: 512-col matmuls filling exactly one PSUM bank
                pcs = []
                for c in range(nchunks):
                    for t in range(NT):
                        pc = psum.tile([P, 16, NJ], F32,
                                       name=f"pc{b}{ot}{r0}_{c}_{t}",
                                       tag=f"ch{(ctag0 + c) % 2}_{t}")
                        pcs.append(pc)
                        for g, (it, dy) in enumerate(
                                (i, d) for i in range(NI) for d in range(3)):
                            lhsT = w_sb[(b, it)][:, dy, t, ot * P:(ot + 1) * P]
                            rs = r0 + c * 16 + dy
                            nc.tensor.matmul(
                                pc[:], lhsT, vt[(b, it)][:, rs:rs + 16, t, :],
                                start=(g == 0), stop=(g == 5),
                            )
                return pcs

            def out_block(b, ot, r0, pcs, ring=None):
                n = len(pcs) // NT
                oh = out_pool.tile([P, 16 * n, W], F32, name=f"oh{b}{ot}{r0}",
                                   tag="oh")
                sub, add = mybir.AluOpType.subtract, mybir.AluOpType.add
                bi = bias_sb[:, ot:ot + 1]
                # ACT drains each t-channel (single PSUM read per op);
                # DVE folds in SBUF only
                for c in range(n):
                    m = pcs[c * NT:(c + 1) * NT]
                    st = out_pool.tile([P, NT, 16, NJ], F32,
                                       name=f"st{b}{ot}{r0}_{c}", tag=f"st{c % 2}")
                    for t in range(NT):
                        nc.scalar.activation(st[:, t], m[t][:], AF.Identity)
                    sq = out_pool.tile([P, 2, 16, NJ], F32,
                                       name=f"sq{b}{ot}{r0}_{c}", tag=f"sq{c % 2}")
                    ov = oh[:, c * 16:(c + 1) * 16, :].rearrange(
                        "p r (j two) -> p r j two", two=2)
                    # out_even = ((m0+m1) + bias) + m2
                    # out_odd  = ((m1-m2) + bias) - m3
                    nc.vector.tensor_tensor(sq[:, 0], st[:, 0], st[:, 1], add)
                    nc.vector.scalar_tensor_tensor(ov[:, :, :, 0], sq[:, 0], bi,
                                                   st[:, 2], add, add)
                    nc.vector.tensor_tensor(sq[:, 1], st[:, 1], st[:, 2], sub)
                    nc.vector.scalar_tensor_tensor(ov[:, :, :, 1], sq[:, 1], bi,
                                                   st[:, 3], add, sub)
                (ring or nc.sync).dma_start(
                    out.ap()[b, ot * P:(ot + 1) * P, r0:r0 + 16 * n, :], oh[:])

            # first two sub-blocks are single 16-row chunks so the conv
            # starts as soon as the first 17-row x slice lands
            out_block(0, 0, 0, mm_block(0, 0, 0, 1, ctag0=0))
            out_block(0, 0, 16, mm_block(0, 0, 16, 1, ctag0=1))
            out_block(0, 0, 32, mm_block(0, 0, 32, 2))

            # sample-1 input transforms (DVE), emitted mid-sample-0 so
            # they are ready well before the b=1 blocks
            for it in range(NI):
                make_v(nc.vector, 1, it, slice(-1, H + 1))

            out_block(0, 1, 0, mm_block(0, 1, 0, 2))
            out_block(0, 1, 32, mm_block(0, 1, 32, 2))
            for half in range(2):
                out_block(1, 0, half * 32, mm_block(1, 0, half * 32, 2))
            out_block(1, 1, 0, mm_block(1, 1, 0, 2))
            # final blocks shrink so the drain tail is short, stores split
            # across both rings
            out_block(1, 1, 32, mm_block(1, 1, 32, 1, ctag0=0), ring=nc.scalar)
            out_block(1, 1, 48, mm_block(1, 1, 48, 1, ctag0=1))
    return nc


_CACHED_NC = None
_PREP_JIT = None


def _get_nc():
    global _CACHED_NC
    if _CACHED_NC is None:
        nc = bacc.Bacc("TRN2", target_bir_lowering=False, debug=False,
                       num_devices=N_CORES)
        build_conv2dmod(nc)
        nc.compile()
        _CACHED_NC = nc
    return _CACHED_NC


def _get_prep():
    """jit'd host-side prep on jax-cpu (multithreaded): x -> bf16 and the
    folded, Winograd-transformed per-sample conv weights
    wu[b,i,dy,t,o] = G @ (w*style*winv) in bf16."""
    global _PREP_JIT
    if _PREP_JIT is None:
        import jax
        import jax.numpy as jnp

        cpu = jax.devices("cpu")[0]

        def _prep(x, y, weights, bias, style_w, style_b):
            style = y @ style_w.T + style_b                       # [B, i]
            w2 = jnp.sum(weights * weights, axis=(2, 3))          # [o, i]
            sigma = (style * style) @ w2.T                        # [B, o]
            winv = 1.0 / jnp.sqrt(sigma + EPS)                    # [B, o]
            wfull = (weights[None] * style[:, None, :, None, None]
                     * winv[:, :, None, None, None])              # [B,o,i,dy,dx]
            w0 = wfull[..., 0]
            w1 = wfull[..., 1]
            w2_ = wfull[..., 2]
            u = jnp.stack([w0, 0.5 * (w0 + w1 + w2_),
                           0.5 * (w0 - w1 + w2_), w2_], axis=-1)  # [B,o,i,dy,t]
            wu = u.transpose(0, 2, 3, 4, 1).astype(jnp.bfloat16)  # [B,i,dy,t,o]
            return x.astype(jnp.bfloat16), wu

        _PREP_JIT = (jax.jit(_prep, device=cpu), cpu)
    return _PREP_JIT


def kernel(x, y, weights, bias, style_w, style_b, _trace=False):
    x = np.asarray(x, dtype=np.float32)
    y = np.asarray(y, dtype=np.float32)
    weights = np.asarray(weights, dtype=np.float32)
    bias = np.asarray(bias, dtype=np.float32)
    style_w = np.asarray(style_w, dtype=np.float32)
    style_b = np.asarray(style_b, dtype=np.float32)

    prep, cpu = _get_prep()
    import jax
    with jax.default_device(cpu):
        xbf, wuj = prep(x, y, weights, bias, style_w, style_b)
        xbf = np.asarray(xbf)
        wun = np.asarray(wuj)
    bias_c = np.ascontiguousarray(bias.reshape(NO, P).T)           # [p, oo]

    nc = _get_nc()
    in_maps = []
    for c in range(N_CORES):
        sl = slice(c * B_LOC, (c + 1) * B_LOC)
        in_maps.append({
            "xbf": np.ascontiguousarray(xbf[sl]),
            "wu": np.ascontiguousarray(wun[sl]),
            "bias_col": bias_c,
        })
    res = bass_utils.run_bass_kernel_spmd(
        nc, in_maps, core_ids=list(range(N_CORES)), trace=_trace
    )
    out = np.concatenate([r["out"] for r in res.results], axis=0)
    if _trace:
        kernel.last_results = res
    return out
